# revision 17
# baseline (speedup 1.0000x reference)
"""Trainium2 Bass kernel for single-head causal attention (B=4, T=4096, C=2048, HS=128).

Sharding: 2 cores per batch element (8 cores, B=4), interleaved 512-row
q-chunks: role A (even cores) owns global chunks [0,2,4,6], role B (odd)
[1,3,5,7]. Each core loads only its own 2048 rows of x (8MB), projects
K^T/V^T for them in all 8 PSUM banks while x streams in, stages them to
DRAM and AllGathers within its pair. The collective is hidden behind the
Q projections and the own-parity half of attention; the partner-parity
half runs after, with both parities of the gathered buffer blended via
per-core 0/1 selectors so all 8 cores run one SPMD graph.

Attention per slot and phase: scores in 2-tile PSUM groups double-
buffered, one batched exp per group on the scalar engine, causal masks
on the vector engine, AV accumulating across the phase in one PSUM bank
(has_written semantics), and the denominator via one full-width
ones-matmul per group on the DVE-summed pair of p-tiles. At each phase
end the AV/den banks fold into SBUF accumulators (2 folds per slot
total). Epilogue: reciprocal_approx_fast + multiply, DMA out y^T; host
transposes.
"""

import math

import numpy as np
import ml_dtypes

import concourse.bacc as bacc
import concourse.tile as tile
from concourse import mybir
from concourse.bass_utils import run_bass_kernel_spmd

B, T, C, HS = 4, 4096, 2048, 128
NCORES = 8
NCT = C // 128           # 16 contraction tiles
TOWN = 2048              # rows owned per core
QTILES_A = [0, 2, 4, 6]
QTILES_B = [1, 3, 5, 7]

BF16 = ml_dtypes.bfloat16


def build_graph(with_collective=True):
    nc = bacc.Bacc(
        "TRN2", target_bir_lowering=False, debug=False, num_devices=NCORES
    )
    bf = mybir.dt.bfloat16
    f32 = mybir.dt.float32
    EXP = mybir.ActivationFunctionType.Exp

    xt_d = nc.dram_tensor("xt", [NCT, 128, TOWN], bf, kind="ExternalInput")
    w3_d = nc.dram_tensor("w3", [128, 3, NCT, HS], bf, kind="ExternalInput")
    mo_d = nc.dram_tensor("mo", [128, 2048], bf, kind="ExternalInput")
    ps0_d = nc.dram_tensor("ps0", [128, 1], f32, kind="ExternalInput")
    ps1_d = nc.dram_tensor("ps1", [128, 1], f32, kind="ExternalInput")
    psz_d = nc.dram_tensor("psz", [128, 1], f32, kind="ExternalInput")
    # y^T per slot, normalized; host transposes to [512, HS]
    out_d = nc.dram_tensor("out", [4, 128, 512], f32, kind="ExternalOutput")

    with tile.TileContext(nc) as tc:
        with (
            tc.tile_pool(name="big", bufs=1) as big,
            tc.tile_pool(name="dram", bufs=1, space="DRAM") as dram,
        ):
            # ---- persistent SBUF tensors ----
            w3 = big.tile([128, 3, NCT, HS], bf, tag="w3")
            xt = big.tile([128, NCT, TOWN], bf, tag="xt")
            mo = big.tile([128, 2048], bf, tag="mo")
            ps0 = big.tile([128, 1], f32, tag="ps0")
            ps1 = big.tile([128, 1], f32, tag="ps1")
            psz = big.tile([128, 1], f32, tag="psz")
            on128 = big.tile([128, 128], bf, tag="on128")
            zr = big.tile([128, 128], bf, tag="zr")
            ktq = big.tile([128, TOWN], bf, tag="ktq")   # own K^T (slot order)
            vtq = big.tile([128, TOWN], bf, tag="vtq")   # own V^T
            v3o = big.tile([128, 16, HS], bf, tag="v3o")  # own V k-major
            ktp = [
                big.tile([128, TOWN], bf, tag=f"ktp{r}", name=f"ktp{r}")
                for r in range(2)
            ]
            v3p = [
                big.tile([128, 16, HS], bf, tag=f"v3p{r}", name=f"v3p{r}")
                for r in range(2)
            ]
            ktpar = big.tile([128, TOWN], bf, tag="ktpar")
            v3par = big.tile([128, 16, HS], bf, tag="v3par")
            qts = [
                big.tile([128, 512], bf, tag=f"qt{s}", name=f"qt{s}")
                for s in range(4)
            ]
            acav = [
                big.tile([128, 512], f32, tag=f"acav{s}", name=f"acav{s}")
                for s in range(4)
            ]
            acdn = [
                big.tile([128, 512], f32, tag=f"acdn{s}", name=f"acdn{s}")
                for s in range(4)
            ]

            kvb = dram.tile([256, TOWN], bf, tag="kvb")
            kvg = dram.tile([512, TOWN], bf, tag="kvg")

            # ---- input DMAs: weights first, then xt round-robin ----
            nc.sync.dma_start(w3[:, :, 0:4, :], w3_d[:, :, 0:4, :])
            qs = [nc.sync, nc.scalar, nc.gpsimd]
            for c in range(NCT):
                qs[c % 3].dma_start(xt[:, c, :], xt_d[c])
                if c == 3:
                    nc.scalar.dma_start(w3[:, :, 4:10, :], w3_d[:, :, 4:10, :])
                elif c == 7:
                    nc.gpsimd.dma_start(
                        w3[:, :, 10:16, :], w3_d[:, :, 10:16, :]
                    )
            nc.gpsimd.dma_start(mo[:], mo_d[:])
            nc.gpsimd.dma_start(ps0[:], ps0_d[:])
            nc.gpsimd.dma_start(ps1[:], ps1_d[:])
            nc.gpsimd.dma_start(psz[:], psz_d[:])

            # ---- constants ----
            nc.vector.memset(zr[:], 0.0)
            nc.vector.memset(on128[:], 1.0)

            # ---- phase A: own K^T and V^T in all 8 PSUM banks while xt
            # streams in (c-outer) ----
            with tc.tile_pool(name="kps", bufs=1, space="PSUM") as kps:
                kbig = kps.tile([128, 8, 512], f32, tag="kbig", name="kbig")
                for c in range(NCT):
                    for wi in range(2):  # 0 = K, 1 = V
                        for t4 in range(4):
                            nc.tensor.matmul(
                                kbig[:, wi * 4 + t4, :],
                                w3[:, wi + 1, c, :],
                                xt[:, c, t4 * 512 : (t4 + 1) * 512],
                                start=(c == 0),
                                stop=(c == NCT - 1),
                            )
                    if 0 < c < NCT - 1:
                        # zero-contribution dummies keep the HAM clock warm
                        # between DMA-paced c-tile bursts
                        for dp in range(3):
                            nc.tensor.matmul(
                                kbig[:, dp, :],
                                zr[:],
                                w3[:, 1, 0:4, :],
                                start=False, stop=False,
                                skip_group_check=True,
                            )
                nc.vector.tensor_copy(ktq[:], kbig[:, 0:4, :])
                nc.vector.tensor_copy(vtq[:], kbig[:, 4:8, :])
                # staging writes, own V transpose, collective
                nc.sync.dma_start(kvb[0:128, :], ktq[:])
                nc.scalar.dma_start(kvb[128:256, :], vtq[:])
                nc.sync.dma_start_transpose(v3o[:], vtq[:])
                if with_collective:
                    nc.gpsimd.collective_compute(
                        "AllGather",
                        mybir.AluOpType.bypass,
                        replica_groups=[[0, 1], [2, 3], [4, 5], [6, 7]],
                        ins=[kvb.opt()],
                        outs=[kvg.opt()],
                    )
                else:  # timeline-model stub: same data volume, no comms
                    nc.scalar.dma_start(kvg[0:256, :], kvb[:])
                    nc.scalar.dma_start(kvg[256:512, :], kvb[:])
                # partner halves (block on the collective)
                nc.gpsimd.dma_start(ktp[0][:], kvg[0:128, :])
                nc.scalar.dma_start(ktp[1][:], kvg[256:384, :])
                nc.sync.dma_start_transpose(v3p[0][:], kvg[128:256, :])
                nc.sync.dma_start_transpose(v3p[1][:], kvg[384:512, :])

            # ---- phase B: Q projections + own attention (hides the
            # collective), then blends + partner attention ----
            with (
                tc.tile_pool(name="srng", bufs=2, space="PSUM") as srng,
                tc.tile_pool(name="acc", bufs=1, space="PSUM") as accp,
                tc.tile_pool(name="fbp", bufs=2, space="PSUM") as fbp,
                tc.tile_pool(name="pp", bufs=4) as pp,
                tc.tile_pool(name="ep", bufs=2) as ep,
            ):
                av = accp.tile([128, 512], f32, tag="av", name="av")
                den = accp.tile([128, 512], f32, tag="den", name="den")

                def q_ops(s):
                    """Yield one op per call: 16 accumulating MMs for Q of
                    slot s into a filler PSUM bank, then the copy-out."""
                    fb = fbp.tile([128, 512], f32, tag="fb", name=f"fbq{s}")
                    for c in range(NCT):
                        def mm(c=c):
                            nc.tensor.matmul(
                                fb[:],
                                w3[:, 0, c, :],
                                xt[:, c, s * 512 : (s + 1) * 512],
                                start=(c == 0),
                                stop=(c == NCT - 1),
                            )
                        yield mm
                    def fin():
                        nc.vector.tensor_copy(qts[s][:], fb[:])
                    yield fin

                def drain(gen):
                    if gen is not None:
                        for op in gen:
                            op()

                def emit_fillers(gen, k):
                    if gen is None:
                        return
                    for _ in range(k):
                        op = next(gen, None)
                        if op is None:
                            return
                        op()

                def emit_blends():
                    nc.vector.tensor_scalar_mul(ktp[0][:], ktp[0][:], ps0[:])
                    nc.vector.tensor_scalar_mul(ktp[1][:], ktp[1][:], ps1[:])
                    nc.vector.tensor_add(ktpar[:], ktp[0][:], ktp[1][:])
                    nc.vector.tensor_scalar_mul(v3p[0][:], v3p[0][:], ps0[:])
                    nc.vector.tensor_scalar_mul(v3p[1][:], v3p[1][:], ps1[:])
                    nc.vector.tensor_add(v3par[:], v3p[0][:], v3p[1][:])

                def emit_slot_phase(s, own, filler):
                    """One phase (own or partner parity) of slot s: E=4(s+1)
                    k-tiles in 2-tile groups; AV/den accumulate in PSUM and
                    fold to SBUF at the end."""
                    kt = ktq if own else ktpar
                    v3 = v3o if own else v3par
                    E = 4 * (s + 1)
                    G = E // 2

                    def emit_scores(g):
                        S = srng.tile([128, 1024], f32, tag="r", name="sg")
                        for i in range(2):
                            t = 2 * g + i
                            nc.tensor.matmul(
                                S[:, i * 512 : (i + 1) * 512],
                                kt[:, t * 128 : (t + 1) * 128],
                                qts[s][:],
                                start=True,
                                stop=True,
                            )
                        p = pp.tile([128, 1024], bf, tag="p", name="pg")
                        nc.scalar.activation(p[:], S[:], EXP)
                        for i in range(2):
                            t = 2 * g + i
                            if t < 4 * s:
                                continue
                            sl = p[:, i * 512 : (i + 1) * 512]
                            if own:  # diagonal chunk: structural mask
                                j = t - 4 * s
                                nc.vector.tensor_mul(
                                    sl, sl, mo[:, j * 512 : (j + 1) * 512]
                                )
                            else:  # last partner chunk: per-core selector
                                nc.vector.tensor_scalar_mul(sl, sl, psz[:])
                        return p

                    def emit_av(g, p):
                        for i in range(2):
                            t = 2 * g + i
                            nc.tensor.matmul(
                                av[:],
                                v3[:, t, :],
                                p[:, i * 512 : (i + 1) * 512],
                                start=(t == 0),
                                stop=(t == E - 1),
                                skip_group_check=True,
                            )
                        ps2 = pp.tile([128, 512], bf, tag="ps2", name="ps2")
                        nc.vector.tensor_add(
                            ps2[:], p[:, 0:512], p[:, 512:1024]
                        )
                        nc.tensor.matmul(
                            den[:],
                            on128[:],
                            ps2[:],
                            start=(g == 0),
                            stop=(g == G - 1),
                            skip_group_check=True,
                        )

                    prev = emit_scores(0)
                    for g in range(1, G):
                        cur = emit_scores(g)
                        emit_av(g - 1, prev)
                        prev = cur
                        emit_fillers(filler, 6)
                    emit_av(G - 1, prev)
                    drain(filler)
                    # fold AV/den banks into the slot's SBUF accumulators
                    if own:
                        nc.vector.tensor_copy(acav[s][:], av[:])
                        nc.vector.tensor_copy(acdn[s][:], den[:])
                    else:
                        nc.vector.tensor_add(acav[s][:], acav[s][:], av[:])
                        nc.vector.tensor_add(acdn[s][:], acdn[s][:], den[:])

                # own phases (hide the collective), Q passes as fillers
                drain(q_ops(0))
                for s in range(4):
                    emit_slot_phase(
                        s, True, q_ops(s + 1) if s + 1 < 4 else None
                    )
                emit_blends()
                # partner phases + per-slot epilogue
                for s in range(4):
                    emit_slot_phase(s, False, None)
                    rb = ep.tile([128, 512], f32, tag="rb", name=f"rb{s}")
                    nc.vector.reciprocal_approx_fast(rb[:], acdn[s][:])
                    ot = ep.tile([128, 512], f32, tag="ot", name=f"ot{s}")
                    nc.vector.tensor_mul(ot[:], acav[s][:], rb[:])
                    nc.gpsimd.dma_start(out_d[s], ot[:])

    nc.compile()
    return nc


def _role_qtiles(h):
    return QTILES_A if h == 0 else QTILES_B


def _diag_mask():
    """[128, 4*512] bf16: tile j of the diagonal 512-chunk, k<=q."""
    m = np.zeros((128, 4, 512), np.float32)
    k = np.arange(128)[:, None]
    q = np.arange(512)[None, :]
    for j in range(4):
        m[:, j, :] = (128 * j + k <= q).astype(np.float32)
    return np.ascontiguousarray(m.reshape(128, 2048)).astype(BF16)


def make_in_maps(x, Wq, Wk, Wv):
    """Host-side sharding + layout prep. x [B,T,C] f32, W* [C,HS] f32."""
    wq_s = np.asarray(Wq, np.float32) / math.sqrt(HS)
    w3 = np.stack(
        [wq_s, np.asarray(Wk, np.float32), np.asarray(Wv, np.float32)]
    )
    w3_arr = np.ascontiguousarray(
        w3.reshape(3, NCT, 128, HS).transpose(2, 0, 1, 3)
    ).astype(BF16)
    mo = _diag_mask()

    in_maps = []
    for core in range(NCORES):
        b, h = core // 2, core % 2
        qtiles = _role_qtiles(h)
        rows = np.concatenate(
            [np.arange(g * 512, (g + 1) * 512) for g in qtiles]
        )
        xr = np.asarray(x[b])[rows]  # [2048 rows, C] f32
        xT = np.ascontiguousarray(xr.T).astype(BF16)  # [C, 2048]
        xt_arr = xT.reshape(NCT, 128, TOWN)  # contiguous c-tiles
        in_maps.append(
            {
                "xt": xt_arr,
                "w3": w3_arr,
                "mo": mo,
                # partner parity selectors: partner parity = 1-h
                "ps0": np.full((128, 1), float(h), np.float32),
                "ps1": np.full((128, 1), float(1 - h), np.float32),
                # partner last chunk: fully masked for role A, visible for B
                "psz": np.full((128, 1), float(h), np.float32),
            }
        )
    return in_maps


def assemble_out(results):
    """results: list of 8 dicts with 'out' [4,128,512] -> y [B,T,HS] f32."""
    y = np.zeros((B, T, HS), np.float32)
    for core in range(NCORES):
        b, h = core // 2, core % 2
        qtiles = _role_qtiles(h)
        o = np.asarray(results[core]["out"])  # [4, 128, 512] = y^T per slot
        for s in range(4):
            g = qtiles[s]
            y[b, g * 512 : (g + 1) * 512] = o[s].T
    return y


_NC_CACHE = None


def _get_graph():
    global _NC_CACHE
    if _NC_CACHE is None:
        _NC_CACHE = build_graph()
    return _NC_CACHE


def kernel(x, Wq, Wk, Wv):
    import time

    nc = _get_graph()
    in_maps = make_in_maps(x, Wq, Wk, Wv)
    try:
        res = run_bass_kernel_spmd(nc, in_maps, list(range(NCORES)))
    except Exception:
        time.sleep(15)  # transient device/mesh hiccup: one retry
        res = run_bass_kernel_spmd(nc, in_maps, list(range(NCORES)))
    return assemble_out(res.results)
